# revision 10
# baseline (speedup 1.0000x reference)
"""Trainium2 Bass kernel: causal self-attention with sigmoid attention and
Bernoulli straight-through sampling (nn_CausalSelfAttention_57002805953253).

Key facts this implementation exploits:

* In the forward pass, the straight-through estimator makes the attention
  matrix numerically EQUAL to the Bernoulli samples (att + (samp - att) ==
  samp exactly in fp32).  Therefore att_var == 0 exactly, att_sum is the
  per-row count of successes, and y is computed from the 0/1 samples.
* The Bernoulli draw is `u < sigmoid(scores)` with u = uniform(key, shape)
  for a FIXED key (input independent).  Since sigmoid is monotonic,
  `u < sigmoid(s)  <=>  s > logit(u)`, so the device kernel only needs the
  raw scores and a precomputed threshold tensor L = logit(u).  The causal
  mask is baked into L as +inf (s > +inf is always false).
* Sharding: core c handles batch b = c//2 and heads hg*8..hg*8+8 (hg=c%2).
  The final projection is computed per-core against the head-slice of
  w_proj; the host adds the two partial results per batch.

Precision strategy: scores (qkv q/k + q@k^T) run as float32r (fp32 data,
FP22 multiply, 2 cycles/row on this silicon) because sample flips scale
with score error.  The value path (v, samples @ v, projection) runs bf16
(1 cycle/row): samples are exactly representable there and the value-path
error is dominated by the sample flips anyway.

Performance notes (from NTFF traces):
* DMA descriptors are one partition-row each and the DGE processes them
  serially (~130 GB/s/queue at 1-4 KB rows), so every large stream is
  host-packed to be partition-contiguous: x/w as [128, k*width] tiles and
  the logit thresholds as one [128, 4608] block per head (18 KB/row).
  DMAs are issued round-robin over sync/scalar/gpsimd queues.
* Even/odd heads' K=64 score matmuls use PE row-groups 0-63/64-127 and
  are emitted adjacently so they execute concurrently.
* qkv for head-pair j+1 is interleaved with attention for pair j to keep
  the PE dense (HAM throttles after ~3.4 us of micro-idling).
"""

import os
import sys
from contextlib import ExitStack

import numpy as np

for _p in ("/opt/trn_rl_repo", "/root/.axon_site/_ro/trn_rl_repo"):
    if os.path.isdir(_p) and _p not in sys.path:
        sys.path.append(_p)

import ml_dtypes  # noqa: E402
import concourse.bass as bass  # noqa: E402,F401
import concourse.tile as tile  # noqa: E402
from concourse import bacc, mybir  # noqa: E402
from concourse.bass_utils import run_bass_kernel_spmd  # noqa: E402

B, T, C, H = 4, 1024, 1024, 16
HS = C // H            # 64 head size
NCORES = 8
HPC = H // 2           # 8 local heads per core
F32 = mybir.dt.float32
F32R = mybir.dt.float32r
BF16 = mybir.dt.bfloat16


def _lt_blocks():
    """Causal block schedule: [(qc, kt, qs, n, boff)] + total columns.

    boff is the column offset of the [128, n] block inside the per-head
    packed [128, 4608] threshold tile.
    """
    blocks = []
    boff = 0
    for qc in range(2):
        for kt in range(4 if qc == 0 else 8):
            qs = max(kt * 128, qc * 512)
            n = qc * 512 + 512 - qs
            blocks.append((qc, kt, qs, n, boff))
            boff += n
    return blocks, boff


LT_BLOCKS, LT_COLS = _lt_blocks()   # LT_COLS = 4608 columns/head

_CACHE = {}


def build_nc():
    """Build the SPMD Bass program (identical on all 8 cores)."""
    nc = bacc.Bacc("TRN2", target_bir_lowering=False, debug=False)

    # all big inputs host-packed partition-contiguous as [128, k*width]
    xt_d = nc.dram_tensor("xt", [128, 8 * T], F32R, kind="ExternalInput")
    wqt_d = nc.dram_tensor("wqt", [128, 8 * 512], F32R, kind="ExternalInput")
    wkt_d = nc.dram_tensor("wkt", [128, 8 * 512], F32R, kind="ExternalInput")
    wvt_d = nc.dram_tensor("wvt", [128, 8 * 512], BF16, kind="ExternalInput")
    wpt_d = nc.dram_tensor("wpt", [128, 4 * C], BF16, kind="ExternalInput")
    ltp_d = nc.dram_tensor("ltp", [HPC, 128, LT_COLS], F32, kind="ExternalInput")
    yp_d = nc.dram_tensor("yp", [T, C], F32, kind="ExternalOutput")
    asum_d = nc.dram_tensor("asum", [HPC, T], F32, kind="ExternalOutput")

    with tile.TileContext(nc) as tc, ExitStack() as ctx:
        persist = ctx.enter_context(tc.tile_pool(name="persist", bufs=1))
        qkpool = ctx.enter_context(tc.tile_pool(name="qkpool", bufs=1))
        ltpool = ctx.enter_context(tc.tile_pool(name="ltpool", bufs=3))
        smpool = ctx.enter_context(tc.tile_pool(name="smpool", bufs=10))
        opool = ctx.enter_context(tc.tile_pool(name="opool", bufs=2))
        astp = ctx.enter_context(tc.tile_pool(name="astp", bufs=2))
        psmm = ctx.enter_context(tc.tile_pool(name="psmm", bufs=4, space="PSUM"))
        psy = ctx.enter_context(tc.tile_pool(name="psy", bufs=4, space="PSUM"))

        engs = [nc.sync, nc.scalar, nc.gpsimd]
        rr = [0]

        def dma(dst, src):
            e = engs[rr[0] % 3]
            rr[0] += 1
            e.dma_start(dst, src)

        # ---- load packed x^T and weights ------------------------------
        xt = persist.tile([128, 8 * T], F32R, tag="xt", name="xt")
        dma(xt[:, 0:4 * T], xt_d[:, 0:4 * T])
        dma(xt[:, 4 * T:8 * T], xt_d[:, 4 * T:8 * T])
        xtb = persist.tile([128, 8 * T], BF16, tag="xtb", name="xtb")
        nc.scalar.copy(xtb[:, 0:4 * T], xt[:, 0:4 * T])
        nc.scalar.copy(xtb[:, 4 * T:8 * T], xt[:, 4 * T:8 * T])

        wq = persist.tile([128, 8 * 512], F32R, tag="wq", name="wq")
        dma(wq[:, :], wqt_d[:, :])
        wk = persist.tile([128, 8 * 512], F32R, tag="wk", name="wk")
        dma(wk[:, :], wkt_d[:, :])
        wv = persist.tile([128, 8 * 512], BF16, tag="wv", name="wv")
        dma(wv[:, :], wvt_d[:, :])
        wp = persist.tile([128, 4 * C], BF16, tag="wp", name="wp")
        dma(wp[:, :], wpt_d[:, :])

        def xts(kc):       # f32r x^T k-tile view [128(C), T]
            return xt[:, kc * T:(kc + 1) * T]

        def xtbs(kc):      # bf16 x^T k-tile view
            return xtb[:, kc * T:(kc + 1) * T]

        def wqs(kc):
            return wq[:, kc * 512:(kc + 1) * 512]

        def wks(kc):
            return wk[:, kc * 512:(kc + 1) * 512]

        def wvs(kc):
            return wv[:, kc * 512:(kc + 1) * 512]

        def wps(jj):
            return wp[:, jj * C:(jj + 1) * C]

        # ---- v first (bf16), augmented with a ones column per head ----
        vaug = [persist.tile([128, HPC * (HS + 1)], BF16,
                             tag=f"va{t}", name=f"va{t}")
                for t in range(8)]
        for tt in range(8):
            ps = psmm.tile([128, 512], F32, tag="psmm", name="ps_v")
            for kc in range(8):
                nc.tensor.matmul(
                    ps[:, :],
                    xtbs(kc)[:, tt * 128:(tt + 1) * 128],
                    wvs(kc),
                    start=(kc == 0), stop=(kc == 7),
                )
            nc.scalar.copy(
                vaug[tt][:, :].rearrange("p (h e) -> p h e", e=HS + 1)[:, :, 0:HS],
                ps[:, :].rearrange("p (h e) -> p h e", e=HS),
            )
            nc.vector.memset(
                vaug[tt][:, :].rearrange("p (h e) -> p h e", e=HS + 1)[:, :, HS:HS + 1],
                1.0,
            )

        # ---- q^T / k^T for one head-pair j (double-buffered over j) ---
        qk_tiles = {}

        def emit_qk(j):
            qt = qkpool.tile([128, T], F32R, tag=f"qt{j % 2}", name=f"qt{j}")
            kt = qkpool.tile([128, T], F32R, tag=f"kt{j % 2}", name=f"kt{j}")
            qk_tiles[j] = (qt, kt)
            for dst, wview in ((qt, wqs), (kt, wks)):
                for half in range(2):
                    ps = psmm.tile([128, 512], F32, tag="psmm", name="ps_qk")
                    for kc in range(8):
                        nc.tensor.matmul(
                            ps[:, :],
                            wview(kc)[:, j * 128:(j + 1) * 128],
                            xts(kc)[:, half * 512:(half + 1) * 512],
                            start=(kc == 0), stop=(kc == 7),
                        )
                    nc.scalar.copy(
                        dst[:, half * 512:(half + 1) * 512], ps[:, :])

        # ---- attention for one head-pair j (heads 2j even, 2j+1 odd) --
        yt_sb = [persist.tile([128, T], BF16, tag=f"yt{j}", name=f"yt{j}")
                 for j in range(4)]

        def emit_att(j):
            qt, kt_t = qk_tiles[j]
            # one packed threshold tile per head: [128, 4608]
            lth = []
            for par in range(2):
                i = 2 * j + par
                lt_t = ltpool.tile([128, LT_COLS], F32, tag="lt",
                                   name=f"lt{i}")
                dma(lt_t[:, :], ltp_d[i])
                lth.append(lt_t)
            for qc in range(2):
                nblk = [blk for blk in LT_BLOCKS if blk[0] == qc]
                first_kt, last_kt = nblk[0][1], nblk[-1][1]
                yps = [psy.tile([HS + 1, 512], F32, tag="psy", name="psy_e"),
                       psy.tile([HS + 1, 512], F32, tag="psy", name="psy_o")]
                for (_, kt, qs, n, boff) in nblk:
                    sps = []
                    for par in range(2):           # even/odd: PE row groups
                        po = 64 * par
                        sp = psmm.tile([128, n], F32, tag="psmm", name="ps_s")
                        nc.tensor.matmul(
                            sp[:, :],
                            kt_t[po:po + 64, kt * 128:(kt + 1) * 128],
                            qt[po:po + 64, qs:qs + n],
                            start=True, stop=True,
                        )
                        sps.append(sp)
                    o = qs - qc * 512
                    for par in range(2):
                        i = 2 * j + par            # local head
                        smp = smpool.tile([128, n], BF16, tag="smp",
                                          name="smp_t")
                        nc.vector.tensor_tensor(
                            smp[:, :], sps[par][:, :],
                            lth[par][:, boff:boff + n],
                            op=mybir.AluOpType.is_gt,
                        )
                        nc.tensor.matmul(
                            yps[par][:, o:o + n],
                            vaug[kt][:, i * (HS + 1):(i + 1) * (HS + 1)],
                            smp[:, :],
                            start=(kt == first_kt), stop=(kt == last_kt),
                        )
                for par in range(2):
                    i = 2 * j + par
                    po = 64 * par
                    nc.scalar.copy(
                        yt_sb[j][po:po + 64, qc * 512:(qc + 1) * 512],
                        yps[par][0:64, :])
                    ast = astp.tile([65, 512], F32, tag="ast", name="ast_t")
                    nc.scalar.copy(ast[64:65, :], yps[par][64:65, :])
                    dma(asum_d[i, qc * 512:(qc + 1) * 512], ast[64:65, :])

        # interleave: qkv(j+1) emitted right after attention(j)
        emit_qk(0)
        for j in range(4):
            if j < 3:
                emit_att(j)
                emit_qk(j + 1)
            else:
                emit_att(j)

        # ---- partial projection y_heads @ wproj_slice^T (bf16) --------
        for tt in range(8):
            ot = opool.tile([128, C], F32, tag="ot", name="ot_t")
            for nch in range(2):
                ps = psmm.tile([128, 512], F32, tag="psmm", name="ps_p")
                for jj in range(4):
                    nc.tensor.matmul(
                        ps[:, :],
                        yt_sb[jj][:, tt * 128:(tt + 1) * 128],
                        wps(jj)[:, nch * 512:(nch + 1) * 512],
                        start=(jj == 0), stop=(jj == 3),
                    )
                nc.scalar.copy(ot[:, nch * 512:(nch + 1) * 512], ps[:, :])
            dma(yp_d[tt * 128:(tt + 1) * 128, :], ot[:, :])

    nc.compile()   # bacc register allocation + finalize before serialization
    return nc


def _get_nc():
    if "nc" not in _CACHE:
        _CACHE["nc"] = build_nc()
    return _CACHE["nc"]


def _get_u():
    """Exactly reproduce the uniform field jax.random.bernoulli draws.

    The container pins jax_default_prng_impl='rbg', whose bitstream is
    backend-dependent — so this must run on the same default device the
    reference uses (do NOT pin to CPU here).
    """
    import jax
    import jax.numpy as jnp
    samp_key = jax.random.fold_in(jax.random.key(0), 42)
    u = jax.random.uniform(samp_key, (B, H, T, T), dtype=jnp.float32)
    return np.asarray(u)


def _prep_ltp():
    """Per-core packed causal logit thresholds: [NCORES, HPC, 128, 4608]."""
    if "ltp" in _CACHE:
        return _CACHE["ltp"]
    u = _get_u()
    ltp = np.empty((NCORES, HPC, 128, LT_COLS), np.float32)
    kk = np.arange(128)[:, None]
    for c in range(NCORES):
        b, hg = c // 2, c % 2
        for i in range(HPC):
            h = hg * HPC + i
            for (qc, kt, qs, n, boff) in LT_BLOCKS:
                ub = u[b, h, qs:qs + n, kt * 128:(kt + 1) * 128].astype(np.float64)
                with np.errstate(divide="ignore"):
                    lt = np.log(ub) - np.log1p(-ub)
                ltb = np.ascontiguousarray(lt.T).astype(np.float32)  # [128, n]
                if qs == kt * 128:
                    ltb[kk > np.arange(n)[None, :]] = np.inf
                ltp[c, i, :, boff:boff + n] = ltb
    _CACHE["ltp"] = ltp
    return ltp


def _pack_rows(a, k):
    """[k*128, W] -> [128, k*W]: partition p holds rows p, 128+p, ..."""
    kk, w = a.shape[0] // 128, a.shape[1]
    assert kk == k
    return np.ascontiguousarray(
        a.reshape(k, 128, w).transpose(1, 0, 2).reshape(128, k * w))


def make_in_maps(x, w_attn, w_proj):
    scale = np.float32(1.0 / np.sqrt(np.float32(HS)))   # 0.125, exact pow2
    ltp = _prep_ltp()
    in_maps = []
    for c in range(NCORES):
        b, hg = c // 2, c % 2
        r0 = hg * HPC * HS
        xt = np.ascontiguousarray(x[b].T)                          # [C, T]
        wqt = np.ascontiguousarray((w_attn[r0:r0 + 512, :] * scale).T)
        wkt = np.ascontiguousarray(w_attn[C + r0:C + r0 + 512, :].T)
        wvt = np.ascontiguousarray(w_attn[2 * C + r0:2 * C + r0 + 512, :].T)
        wpt = np.ascontiguousarray(w_proj[:, r0:r0 + 512].T)       # [512, C]
        in_maps.append({
            "xt": _pack_rows(xt, 8),
            "wqt": _pack_rows(wqt, 8),
            "wkt": _pack_rows(wkt, 8),
            "wvt": _pack_rows(wvt, 8).astype(ml_dtypes.bfloat16),
            "wpt": _pack_rows(wpt, 4).astype(ml_dtypes.bfloat16),
            "ltp": ltp[c],
        })
    return in_maps


def assemble(results):
    """Combine per-core outputs into full outputs."""
    y = np.zeros((B, T, C), np.float32)
    att_sum = np.zeros((B, H, T), np.float32)
    for c in range(NCORES):
        b, hg = c // 2, c % 2
        y[b] += results[c]["yp"]
        att_sum[b, hg * HPC:(hg + 1) * HPC, :] = results[c]["asum"]
    att_var = np.zeros((B, H, T), np.float32)
    return y, att_sum, att_var


def kernel(x, w_attn, w_proj, **run_kwargs):
    x = np.asarray(x, dtype=np.float32)
    w_attn = np.asarray(w_attn, dtype=np.float32)
    w_proj = np.asarray(w_proj, dtype=np.float32)
    nc = _get_nc()
    in_maps = make_in_maps(x, w_attn, w_proj)
    res = run_bass_kernel_spmd(nc, in_maps, core_ids=list(range(NCORES)),
                               **run_kwargs)
    out = assemble(res.results)
    _CACHE["last_result"] = res
    return out


# revision 11
# speedup vs baseline: 1.0673x; 1.0673x over previous
"""Trainium2 Bass kernel: causal self-attention with sigmoid attention and
Bernoulli straight-through sampling (nn_CausalSelfAttention_57002805953253).

Key facts this implementation exploits:

* In the forward pass, the straight-through estimator makes the attention
  matrix numerically EQUAL to the Bernoulli samples (att + (samp - att) ==
  samp exactly in fp32).  Therefore att_var == 0 exactly, att_sum is the
  per-row count of successes, and y is computed from the 0/1 samples.
* The Bernoulli draw is `u < sigmoid(scores)` with u = uniform(key, shape)
  for a FIXED key (input independent).  Since sigmoid is monotonic,
  `u < sigmoid(s)  <=>  s > logit(u)`, so the device kernel only needs the
  raw scores and a precomputed threshold tensor L = logit(u).  The causal
  mask is baked into L as +inf (s > +inf is always false).
* Sharding: core c handles batch b = c//2 and heads hg*8..hg*8+8 (hg=c%2).
  The final projection is computed per-core against the head-slice of
  w_proj; the host adds the two partial results per batch.

Precision strategy: scores (qkv q/k + q@k^T) run as float32r (fp32 data,
FP22 multiply, 2 cycles/row on this silicon) because sample flips scale
with score error.  The value path (v, samples @ v, projection) runs bf16
(1 cycle/row): samples are exactly representable there and the value-path
error is dominated by the sample flips anyway.

Performance notes (from NTFF traces):
* The PE executes its stream in order, so the kernel is emitted as one
  software-pipelined schedule: per causal block m, score-pair(m) then the
  DVE compare(m) then y-pair(m-2), with one qkv/v accumulation group
  injected per slot so the PE never idles on the compare/DMA latency and
  HAM stays un-throttled.
* DMA descriptors are one partition-row each and the DGE processes them
  serially (~130 GB/s/queue at small rows), so all large streams are
  host-packed partition-contiguous: x/w as [128, k*width] and the logit
  thresholds as one [128, 4608] tile per head (18 KB rows).  Issues are
  round-robined over the sync/scalar/gpsimd queues, per k-tile so the
  first qkv accumulations start as soon as the first tiles land.
* Even/odd heads' K=64 score matmuls use PE row-groups 0-63/64-127 and
  are emitted adjacently so they execute concurrently.
"""

import os
import sys
from contextlib import ExitStack

import numpy as np

for _p in ("/opt/trn_rl_repo", "/root/.axon_site/_ro/trn_rl_repo"):
    if os.path.isdir(_p) and _p not in sys.path:
        sys.path.append(_p)

import ml_dtypes  # noqa: E402
import concourse.bass as bass  # noqa: E402,F401
import concourse.tile as tile  # noqa: E402
from concourse import bacc, mybir  # noqa: E402
from concourse.bass_utils import run_bass_kernel_spmd  # noqa: E402

B, T, C, H = 4, 1024, 1024, 16
HS = C // H            # 64 head size
NCORES = 8
HPC = H // 2           # 8 local heads per core
F32 = mybir.dt.float32
F32R = mybir.dt.float32r
BF16 = mybir.dt.bfloat16


def _lt_blocks():
    """Flat causal block schedule: [(qc, kt, qs, n, boff)] + total columns."""
    blocks = []
    boff = 0
    for qc in range(2):
        for kt in range(4 if qc == 0 else 8):
            qs = max(kt * 128, qc * 512)
            n = qc * 512 + 512 - qs
            blocks.append((qc, kt, qs, n, boff))
            boff += n
    return blocks, boff


LT_BLOCKS, LT_COLS = _lt_blocks()   # 12 blocks, LT_COLS = 4608 columns/head

_CACHE = {}


def build_nc():
    """Build the SPMD Bass program (identical on all 8 cores)."""
    nc = bacc.Bacc("TRN2", target_bir_lowering=False, debug=False)

    # all big inputs host-packed partition-contiguous as [128, k*width]
    xt_d = nc.dram_tensor("xt", [128, 8 * T], F32R, kind="ExternalInput")
    wqt_d = nc.dram_tensor("wqt", [128, 8 * 512], F32R, kind="ExternalInput")
    wkt_d = nc.dram_tensor("wkt", [128, 8 * 512], F32R, kind="ExternalInput")
    wvt_d = nc.dram_tensor("wvt", [128, 8 * 512], BF16, kind="ExternalInput")
    wpt_d = nc.dram_tensor("wpt", [128, 4 * C], BF16, kind="ExternalInput")
    ltp_d = nc.dram_tensor("ltp", [HPC, 128, LT_COLS], F32, kind="ExternalInput")
    yp_d = nc.dram_tensor("yp", [T, C], F32, kind="ExternalOutput")
    asum_d = nc.dram_tensor("asum", [HPC, T], F32, kind="ExternalOutput")

    with tile.TileContext(nc) as tc, ExitStack() as ctx:
        persist = ctx.enter_context(tc.tile_pool(name="persist", bufs=1))
        qkpool = ctx.enter_context(tc.tile_pool(name="qkpool", bufs=1))
        ltpool = ctx.enter_context(tc.tile_pool(name="ltpool", bufs=3))
        smpool = ctx.enter_context(tc.tile_pool(name="smpool", bufs=10))
        opool = ctx.enter_context(tc.tile_pool(name="opool", bufs=2))
        astp = ctx.enter_context(tc.tile_pool(name="astp", bufs=2))
        psa = ctx.enter_context(tc.tile_pool(name="psa", bufs=1, space="PSUM"))
        pss = ctx.enter_context(tc.tile_pool(name="pss", bufs=4, space="PSUM"))
        psy = ctx.enter_context(tc.tile_pool(name="psy", bufs=3, space="PSUM"))

        engs = [nc.sync, nc.scalar, nc.gpsimd]
        rr = [0]

        def dma(dst, src):
            e = engs[rr[0] % 3]
            rr[0] += 1
            e.dma_start(dst, src)

        # ---- load packed x^T and weights, per k-tile ------------------
        xt = persist.tile([128, 8 * T], F32R, tag="xt", name="xt")
        xtb = persist.tile([128, 8 * T], BF16, tag="xtb", name="xtb")
        wq = persist.tile([128, 8 * 512], F32R, tag="wq", name="wq")
        wk = persist.tile([128, 8 * 512], F32R, tag="wk", name="wk")
        wv = persist.tile([128, 8 * 512], BF16, tag="wv", name="wv")
        wp = persist.tile([128, 4 * C], BF16, tag="wp", name="wp")

        for kc in range(8):
            dma(wv[:, kc * 512:(kc + 1) * 512],
                wvt_d[:, kc * 512:(kc + 1) * 512])
            dma(xt[:, kc * T:(kc + 1) * T], xt_d[:, kc * T:(kc + 1) * T])
            nc.scalar.copy(xtb[:, kc * T:(kc + 1) * T],
                           xt[:, kc * T:(kc + 1) * T])
            dma(wq[:, kc * 512:(kc + 1) * 512],
                wqt_d[:, kc * 512:(kc + 1) * 512])
            dma(wk[:, kc * 512:(kc + 1) * 512],
                wkt_d[:, kc * 512:(kc + 1) * 512])
        dma(wp[:, :], wpt_d[:, :])

        def xts(kc):
            return xt[:, kc * T:(kc + 1) * T]

        def xtbs(kc):
            return xtb[:, kc * T:(kc + 1) * T]

        def wqs(kc):
            return wq[:, kc * 512:(kc + 1) * 512]

        def wks(kc):
            return wk[:, kc * 512:(kc + 1) * 512]

        def wvs(kc):
            return wv[:, kc * 512:(kc + 1) * 512]

        def wps(jj):
            return wp[:, jj * C:(jj + 1) * C]

        # ---- persistent result tiles ----------------------------------
        vaug = [persist.tile([128, HPC * (HS + 1)], BF16,
                             tag=f"va{t}", name=f"va{t}")
                for t in range(8)]
        yt_sb = [persist.tile([128, T], BF16, tag=f"yt{j}", name=f"yt{j}")
                 for j in range(4)]
        qk_tiles = {}

        # ---- PE work-group emitters (each = one accumulation group) ---
        def emit_v_group(tt):
            ps = psa.tile([128, 512], F32, tag="psa", name="ps_v")
            for kc in range(8):
                nc.tensor.matmul(
                    ps[:, :],
                    xtbs(kc)[:, tt * 128:(tt + 1) * 128],
                    wvs(kc),
                    start=(kc == 0), stop=(kc == 7),
                )
            nc.scalar.copy(
                vaug[tt][:, :].rearrange("p (h e) -> p h e", e=HS + 1)[:, :, 0:HS],
                ps[:, :].rearrange("p (h e) -> p h e", e=HS),
            )
            nc.vector.memset(
                vaug[tt][:, :].rearrange("p (h e) -> p h e", e=HS + 1)[:, :, HS:HS + 1],
                1.0,
            )

        def alloc_qk(j):
            qt = qkpool.tile([128, T], F32R, tag=f"qt{j % 2}", name=f"qt{j}")
            kt = qkpool.tile([128, T], F32R, tag=f"kt{j % 2}", name=f"kt{j}")
            qk_tiles[j] = (qt, kt)

        def emit_qk_group(j, which, half):
            qt, kt = qk_tiles[j]
            dst, wview = (qt, wqs) if which == 0 else (kt, wks)
            ps = psa.tile([128, 512], F32, tag="psa", name="ps_qk")
            for kc in range(8):
                nc.tensor.matmul(
                    ps[:, :],
                    wview(kc)[:, j * 128:(j + 1) * 128],
                    xts(kc)[:, half * 512:(half + 1) * 512],
                    start=(kc == 0), stop=(kc == 7),
                )
            nc.scalar.copy(dst[:, half * 512:(half + 1) * 512], ps[:, :])

        # ---- software-pipelined attention for head-pair j -------------
        LAG = 2

        def emit_pair(j, inject):
            """inject: list of callables; at most one is emitted per slot."""
            qt, kt_t = qk_tiles[j]
            lth = []
            for par in range(2):
                i = 2 * j + par
                lt_t = ltpool.tile([128, LT_COLS], F32, tag="lt",
                                   name=f"lt{i}")
                dma(lt_t[:, :], ltp_d[i])
                lth.append(lt_t)
            nblocks = len(LT_BLOCKS)          # 12
            yps_by_qc = {}
            smp_tiles = [None] * nblocks      # per block: (smp_e, smp_o)
            inj = list(inject)

            def qc_of(m):
                return LT_BLOCKS[m][0]

            def emit_score_and_compare(m):
                (qc, kt, qs, n, boff) = LT_BLOCKS[m]
                if qc not in yps_by_qc:
                    yps_by_qc[qc] = [
                        psy.tile([HS + 1, 512], F32, tag="psy", name="psy_e"),
                        psy.tile([HS + 1, 512], F32, tag="psy", name="psy_o")]
                sps = []
                for par in range(2):
                    po = 64 * par
                    sp = pss.tile([128, n], F32, tag="pss", name="ps_s")
                    nc.tensor.matmul(
                        sp[:, :],
                        kt_t[po:po + 64, kt * 128:(kt + 1) * 128],
                        qt[po:po + 64, qs:qs + n],
                        start=True, stop=True,
                    )
                    sps.append(sp)
                pair_smp = []
                for par in range(2):
                    smp = smpool.tile([128, n], BF16, tag="smp", name="smp_t")
                    nc.vector.tensor_tensor(
                        smp[:, :], sps[par][:, :],
                        lth[par][:, boff:boff + n],
                        op=mybir.AluOpType.is_gt,
                    )
                    pair_smp.append(smp)
                smp_tiles[m] = pair_smp

            def emit_y(m):
                (qc, kt, qs, n, boff) = LT_BLOCKS[m]
                first_kt = 0
                last_kt = 3 if qc == 0 else 7
                o = qs - qc * 512
                for par in range(2):
                    i = 2 * j + par
                    nc.tensor.matmul(
                        yps_by_qc[qc][par][:, o:o + n],
                        vaug[kt][:, i * (HS + 1):(i + 1) * (HS + 1)],
                        smp_tiles[m][par][:, :],
                        start=(kt == first_kt), stop=(kt == last_kt),
                    )
                if kt == last_kt:             # qc complete -> drain psums
                    for par in range(2):
                        i = 2 * j + par
                        po = 64 * par
                        nc.scalar.copy(
                            yt_sb[j][po:po + 64, qc * 512:(qc + 1) * 512],
                            yps_by_qc[qc][par][0:64, :])
                        ast = astp.tile([65, 512], F32, tag="ast",
                                        name="ast_t")
                        nc.scalar.copy(ast[64:65, :],
                                       yps_by_qc[qc][par][64:65, :])
                        dma(asum_d[i, qc * 512:(qc + 1) * 512], ast[64:65, :])

            for m in range(nblocks + LAG):
                if m < nblocks:
                    emit_score_and_compare(m)
                if m - LAG >= 0:
                    emit_y(m - LAG)
                if m < len(inj):
                    inj[m]()

        # ---- full schedule --------------------------------------------
        alloc_qk(0)
        for which in range(2):
            for half in range(2):
                emit_qk_group(0, which, half)

        for j in range(4):
            inject = []
            if j == 0:
                inject += [lambda tt=tt: emit_v_group(tt) for tt in range(8)]
            if j < 3:
                alloc_qk(j + 1)
                inject += [
                    lambda jj=j + 1, w=w, h=h: emit_qk_group(jj, w, h)
                    for w in range(2) for h in range(2)]
            emit_pair(j, inject)

        # ---- partial projection y_heads @ wproj_slice^T (bf16) --------
        for tt in range(8):
            ot = opool.tile([128, C], F32, tag="ot", name="ot_t")
            for nch in range(2):
                ps = pss.tile([128, 512], F32, tag="pss", name="ps_p")
                for jj in range(4):
                    nc.tensor.matmul(
                        ps[:, :],
                        yt_sb[jj][:, tt * 128:(tt + 1) * 128],
                        wps(jj)[:, nch * 512:(nch + 1) * 512],
                        start=(jj == 0), stop=(jj == 3),
                    )
                nc.scalar.copy(ot[:, nch * 512:(nch + 1) * 512], ps[:, :])
            dma(yp_d[tt * 128:(tt + 1) * 128, :], ot[:, :])

    nc.compile()   # bacc register allocation + finalize before serialization
    return nc


def _get_nc():
    if "nc" not in _CACHE:
        _CACHE["nc"] = build_nc()
    return _CACHE["nc"]


def _get_u():
    """Exactly reproduce the uniform field jax.random.bernoulli draws.

    The container pins jax_default_prng_impl='rbg', whose bitstream is
    backend-dependent — so this must run on the same default device the
    reference uses (do NOT pin to CPU here).
    """
    import jax
    import jax.numpy as jnp
    samp_key = jax.random.fold_in(jax.random.key(0), 42)
    u = jax.random.uniform(samp_key, (B, H, T, T), dtype=jnp.float32)
    return np.asarray(u)


def _prep_ltp():
    """Per-core packed causal logit thresholds: [NCORES, HPC, 128, 4608]."""
    if "ltp" in _CACHE:
        return _CACHE["ltp"]
    u = _get_u()
    ltp = np.empty((NCORES, HPC, 128, LT_COLS), np.float32)
    kk = np.arange(128)[:, None]
    for c in range(NCORES):
        b, hg = c // 2, c % 2
        for i in range(HPC):
            h = hg * HPC + i
            for (qc, kt, qs, n, boff) in LT_BLOCKS:
                ub = u[b, h, qs:qs + n, kt * 128:(kt + 1) * 128].astype(np.float64)
                with np.errstate(divide="ignore"):
                    lt = np.log(ub) - np.log1p(-ub)
                ltb = np.ascontiguousarray(lt.T).astype(np.float32)  # [128, n]
                if qs == kt * 128:
                    ltb[kk > np.arange(n)[None, :]] = np.inf
                ltp[c, i, :, boff:boff + n] = ltb
    _CACHE["ltp"] = ltp
    return ltp


def _pack_rows(a, k):
    """[k*128, W] -> [128, k*W]: partition p holds rows p, 128+p, ..."""
    kk, w = a.shape[0] // 128, a.shape[1]
    assert kk == k
    return np.ascontiguousarray(
        a.reshape(k, 128, w).transpose(1, 0, 2).reshape(128, k * w))


def make_in_maps(x, w_attn, w_proj):
    scale = np.float32(1.0 / np.sqrt(np.float32(HS)))   # 0.125, exact pow2
    ltp = _prep_ltp()
    in_maps = []
    for c in range(NCORES):
        b, hg = c // 2, c % 2
        r0 = hg * HPC * HS
        xt = np.ascontiguousarray(x[b].T)                          # [C, T]
        wqt = np.ascontiguousarray((w_attn[r0:r0 + 512, :] * scale).T)
        wkt = np.ascontiguousarray(w_attn[C + r0:C + r0 + 512, :].T)
        wvt = np.ascontiguousarray(w_attn[2 * C + r0:2 * C + r0 + 512, :].T)
        wpt = np.ascontiguousarray(w_proj[:, r0:r0 + 512].T)       # [512, C]
        in_maps.append({
            "xt": _pack_rows(xt, 8),
            "wqt": _pack_rows(wqt, 8),
            "wkt": _pack_rows(wkt, 8),
            "wvt": _pack_rows(wvt, 8).astype(ml_dtypes.bfloat16),
            "wpt": _pack_rows(wpt, 4).astype(ml_dtypes.bfloat16),
            "ltp": ltp[c],
        })
    return in_maps


def assemble(results):
    """Combine per-core outputs into full outputs."""
    y = np.zeros((B, T, C), np.float32)
    att_sum = np.zeros((B, H, T), np.float32)
    for c in range(NCORES):
        b, hg = c // 2, c % 2
        y[b] += results[c]["yp"]
        att_sum[b, hg * HPC:(hg + 1) * HPC, :] = results[c]["asum"]
    att_var = np.zeros((B, H, T), np.float32)
    return y, att_sum, att_var


def kernel(x, w_attn, w_proj, **run_kwargs):
    x = np.asarray(x, dtype=np.float32)
    w_attn = np.asarray(w_attn, dtype=np.float32)
    w_proj = np.asarray(w_proj, dtype=np.float32)
    nc = _get_nc()
    in_maps = make_in_maps(x, w_attn, w_proj)
    res = run_bass_kernel_spmd(nc, in_maps, core_ids=list(range(NCORES)),
                               **run_kwargs)
    out = assemble(res.results)
    _CACHE["last_result"] = res
    return out


# revision 13
# speedup vs baseline: 1.1069x; 1.0371x over previous
"""Trainium2 Bass kernel: causal self-attention with sigmoid attention and
Bernoulli straight-through sampling (nn_CausalSelfAttention_57002805953253).

Key facts this implementation exploits:

* In the forward pass, the straight-through estimator makes the attention
  matrix numerically EQUAL to the Bernoulli samples (att + (samp - att) ==
  samp exactly in fp32).  Therefore att_var == 0 exactly, att_sum is the
  per-row count of successes, and y is computed from the 0/1 samples.
* The Bernoulli draw is `u < sigmoid(scores)` with u = uniform(key, shape)
  for a FIXED key (input independent).  Since sigmoid is monotonic,
  `u < sigmoid(s)  <=>  s > logit(u)`, so the device kernel only needs the
  raw scores and a precomputed threshold tensor L = logit(u).  The causal
  mask is baked into L as +inf (s > +inf is always false).
* Sharding: core c handles batch b = c//2 and heads hg*8..hg*8+8 (hg=c%2).
  The final projection is computed per-core against the head-slice of
  w_proj; the host adds the two partial results per batch.

Precision strategy: scores (qkv q/k + q@k^T) run as float32r (fp32 data,
FP22 multiply, 2 cycles/row on this silicon) because sample flips scale
with score error.  The value path (v, samples @ v, projection) runs bf16
(1 cycle/row): samples are exactly representable there and the value-path
error is dominated by the sample flips anyway.

Performance notes (from NTFF traces):
* The PE executes its stream in order, so the kernel is emitted as one
  software-pipelined schedule: per causal block m, score-pair(m) then the
  DVE compare(m) then y-pair(m-2), with one qkv/v accumulation group
  injected per slot so the PE never idles on the compare/DMA latency and
  HAM stays un-throttled.
* DMA descriptors are one partition-row each and the DGE processes them
  serially (~130 GB/s/queue at small rows), so all large streams are
  host-packed partition-contiguous: x/w as [128, k*width] and the logit
  thresholds as one [128, 4608] tile per head (18 KB rows).  Issues are
  round-robined over the sync/scalar/gpsimd queues, per k-tile so the
  first qkv accumulations start as soon as the first tiles land.
* Even/odd heads' K=64 score matmuls use PE row-groups 0-63/64-127 and
  are emitted adjacently so they execute concurrently.
"""

import os
import sys
from contextlib import ExitStack

import numpy as np

for _p in ("/opt/trn_rl_repo", "/root/.axon_site/_ro/trn_rl_repo"):
    if os.path.isdir(_p) and _p not in sys.path:
        sys.path.append(_p)

import ml_dtypes  # noqa: E402
import concourse.bass as bass  # noqa: E402,F401
import concourse.tile as tile  # noqa: E402
from concourse import bacc, mybir  # noqa: E402
from concourse.bass_utils import run_bass_kernel_spmd  # noqa: E402

B, T, C, H = 4, 1024, 1024, 16
HS = C // H            # 64 head size
NCORES = 8
HPC = H // 2           # 8 local heads per core
F32 = mybir.dt.float32
F32R = mybir.dt.float32r
BF16 = mybir.dt.bfloat16


def _lt_blocks():
    """Flat causal block schedule: [(qc, kt, qs, n, boff)] + total columns."""
    blocks = []
    boff = 0
    for qc in range(2):
        for kt in range(4 if qc == 0 else 8):
            qs = max(kt * 128, qc * 512)
            n = qc * 512 + 512 - qs
            blocks.append((qc, kt, qs, n, boff))
            boff += n
    return blocks, boff


LT_BLOCKS, LT_COLS = _lt_blocks()   # 12 blocks, LT_COLS = 4608 columns/head

_CACHE = {}


def build_nc():
    """Build the SPMD Bass program (identical on all 8 cores)."""
    nc = bacc.Bacc("TRN2", target_bir_lowering=False, debug=False)

    # all big inputs host-packed partition-contiguous as [128, k*width]
    xt_d = nc.dram_tensor("xt", [128, 8 * T], F32R, kind="ExternalInput")
    wqt_d = nc.dram_tensor("wqt", [128, 8 * 512], F32R, kind="ExternalInput")
    wkt_d = nc.dram_tensor("wkt", [128, 8 * 512], F32R, kind="ExternalInput")
    wvt_d = nc.dram_tensor("wvt", [128, 8 * 512], BF16, kind="ExternalInput")
    wpt_d = nc.dram_tensor("wpt", [128, 4 * C], BF16, kind="ExternalInput")
    ltp_d = nc.dram_tensor("ltp", [HPC, 128, LT_COLS], F32, kind="ExternalInput")
    yp_d = nc.dram_tensor("yp", [T, C], F32, kind="ExternalOutput")
    asum_d = nc.dram_tensor("asum", [HPC, T], F32, kind="ExternalOutput")

    with tile.TileContext(nc) as tc, ExitStack() as ctx:
        persist = ctx.enter_context(tc.tile_pool(name="persist", bufs=1))
        qkpool = ctx.enter_context(tc.tile_pool(name="qkpool", bufs=1))
        ltp0 = ctx.enter_context(tc.tile_pool(name="ltp0", bufs=3))
        ltp1 = ctx.enter_context(tc.tile_pool(name="ltp1", bufs=3))
        smpool = ctx.enter_context(tc.tile_pool(name="smpool", bufs=8))
        opool = ctx.enter_context(tc.tile_pool(name="opool", bufs=2))
        astp = ctx.enter_context(tc.tile_pool(name="astp", bufs=2))
        psa = ctx.enter_context(tc.tile_pool(name="psa", bufs=1, space="PSUM"))
        pss = ctx.enter_context(tc.tile_pool(name="pss", bufs=4, space="PSUM"))
        psy = ctx.enter_context(tc.tile_pool(name="psy", bufs=3, space="PSUM"))

        engs = [nc.sync, nc.scalar, nc.gpsimd]
        rr = [0]

        def dma(dst, src):
            e = engs[rr[0] % 3]
            rr[0] += 1
            e.dma_start(dst, src)

        # ---- load packed x^T and weights, per k-tile ------------------
        xt = persist.tile([128, 8 * T], F32R, tag="xt", name="xt")
        xtb = persist.tile([128, 8 * T], BF16, tag="xtb", name="xtb")
        wq = persist.tile([128, 8 * 512], F32R, tag="wq", name="wq")
        wk = persist.tile([128, 8 * 512], F32R, tag="wk", name="wk")
        wv = persist.tile([128, 8 * 512], BF16, tag="wv", name="wv")
        wp = persist.tile([128, 4 * C], BF16, tag="wp", name="wp")

        for kc in range(8):
            dma(wv[:, kc * 512:(kc + 1) * 512],
                wvt_d[:, kc * 512:(kc + 1) * 512])
            dma(xt[:, kc * T:(kc + 1) * T], xt_d[:, kc * T:(kc + 1) * T])
            nc.scalar.copy(xtb[:, kc * T:(kc + 1) * T],
                           xt[:, kc * T:(kc + 1) * T])
            dma(wq[:, kc * 512:(kc + 1) * 512],
                wqt_d[:, kc * 512:(kc + 1) * 512])
            dma(wk[:, kc * 512:(kc + 1) * 512],
                wkt_d[:, kc * 512:(kc + 1) * 512])

        def xts(kc):
            return xt[:, kc * T:(kc + 1) * T]

        def xtbs(kc):
            return xtb[:, kc * T:(kc + 1) * T]

        def wqs(kc):
            return wq[:, kc * 512:(kc + 1) * 512]

        def wks(kc):
            return wk[:, kc * 512:(kc + 1) * 512]

        def wvs(kc):
            return wv[:, kc * 512:(kc + 1) * 512]

        def wps(jj):
            return wp[:, jj * C:(jj + 1) * C]

        # ---- persistent result tiles ----------------------------------
        vaug = [persist.tile([128, HPC * (HS + 1)], BF16,
                             tag=f"va{t}", name=f"va{t}")
                for t in range(8)]
        yt_sb = [persist.tile([128, T], BF16, tag=f"yt{j}", name=f"yt{j}")
                 for j in range(4)]
        qk_tiles = {}

        # ---- PE work-group emitters (each = one accumulation group) ---
        def emit_v_group(tt):
            ps = psa.tile([128, 512], F32, tag="psa", name="ps_v")
            for kc in range(8):
                nc.tensor.matmul(
                    ps[:, :],
                    xtbs(kc)[:, tt * 128:(tt + 1) * 128],
                    wvs(kc),
                    start=(kc == 0), stop=(kc == 7),
                )
            nc.scalar.copy(
                vaug[tt][:, :].rearrange("p (h e) -> p h e", e=HS + 1)[:, :, 0:HS],
                ps[:, :].rearrange("p (h e) -> p h e", e=HS),
            )
            nc.vector.memset(
                vaug[tt][:, :].rearrange("p (h e) -> p h e", e=HS + 1)[:, :, HS:HS + 1],
                1.0,
            )

        def alloc_qk(j):
            qt = qkpool.tile([128, T], F32R, tag=f"qt{j % 2}", name=f"qt{j}")
            kt = qkpool.tile([128, T], F32R, tag=f"kt{j % 2}", name=f"kt{j}")
            qk_tiles[j] = (qt, kt)

        def emit_qk_group(j, which, half):
            qt, kt = qk_tiles[j]
            dst, wview = (qt, wqs) if which == 0 else (kt, wks)
            ps = psa.tile([128, 512], F32, tag="psa", name="ps_qk")
            for kc in range(8):
                nc.tensor.matmul(
                    ps[:, :],
                    wview(kc)[:, j * 128:(j + 1) * 128],
                    xts(kc)[:, half * 512:(half + 1) * 512],
                    start=(kc == 0), stop=(kc == 7),
                )
            nc.scalar.copy(dst[:, half * 512:(half + 1) * 512], ps[:, :])

        # ---- software-pipelined attention for head-pair j -------------
        LAG = 2

        def emit_pair(j, inject):
            """inject: list of callables; at most one is emitted per slot."""
            qt, kt_t = qk_tiles[j]
            lth = {}            # (par, qc) -> (tile, col_base)
            for par in range(2):
                i = 2 * j + par
                a = ltp0.tile([128, 1280], F32, tag="lt0", name=f"lt0_{i}")
                dma(a[:, :], ltp_d[i, :, 0:1280])
                lth[(par, 0)] = (a, 0)
            for par in range(2):
                i = 2 * j + par
                b = ltp1.tile([128, LT_COLS - 1280], F32, tag="lt1",
                              name=f"lt1_{i}")
                dma(b[:, :], ltp_d[i, :, 1280:LT_COLS])
                lth[(par, 1)] = (b, 1280)
            nblocks = len(LT_BLOCKS)          # 12
            yps_by_qc = {}
            smp_tiles = [None] * nblocks      # per block: (smp_e, smp_o)
            inj = list(inject)

            def qc_of(m):
                return LT_BLOCKS[m][0]

            def emit_score_and_compare(m):
                (qc, kt, qs, n, boff) = LT_BLOCKS[m]
                if qc not in yps_by_qc:
                    yps_by_qc[qc] = [
                        psy.tile([HS + 1, 512], F32, tag="psy", name="psy_e"),
                        psy.tile([HS + 1, 512], F32, tag="psy", name="psy_o")]
                sps = []
                for par in range(2):
                    po = 64 * par
                    sp = pss.tile([128, n], F32, tag="pss", name="ps_s")
                    nc.tensor.matmul(
                        sp[:, :],
                        kt_t[po:po + 64, kt * 128:(kt + 1) * 128],
                        qt[po:po + 64, qs:qs + n],
                        start=True, stop=True,
                    )
                    sps.append(sp)
                pair_smp = []
                for par in range(2):
                    lt_t, cbase = lth[(par, qc)]
                    smp = smpool.tile([128, n], BF16, tag="smp", name="smp_t")
                    nc.vector.tensor_tensor(
                        smp[:, :], sps[par][:, :],
                        lt_t[:, boff - cbase:boff - cbase + n],
                        op=mybir.AluOpType.is_gt,
                    )
                    pair_smp.append(smp)
                smp_tiles[m] = pair_smp

            def emit_y(m):
                (qc, kt, qs, n, boff) = LT_BLOCKS[m]
                first_kt = 0
                last_kt = 3 if qc == 0 else 7
                o = qs - qc * 512
                for par in range(2):
                    i = 2 * j + par
                    nc.tensor.matmul(
                        yps_by_qc[qc][par][:, o:o + n],
                        vaug[kt][:, i * (HS + 1):(i + 1) * (HS + 1)],
                        smp_tiles[m][par][:, :],
                        start=(kt == first_kt), stop=(kt == last_kt),
                    )
                if kt == last_kt:             # qc complete -> drain psums
                    for par in range(2):
                        i = 2 * j + par
                        po = 64 * par
                        nc.scalar.copy(
                            yt_sb[j][po:po + 64, qc * 512:(qc + 1) * 512],
                            yps_by_qc[qc][par][0:64, :])
                        ast = astp.tile([65, 512], F32, tag="ast",
                                        name="ast_t")
                        nc.scalar.copy(ast[64:65, :],
                                       yps_by_qc[qc][par][64:65, :])
                        dma(asum_d[i, qc * 512:(qc + 1) * 512], ast[64:65, :])

            for m in range(nblocks + LAG):
                if m < nblocks:
                    emit_score_and_compare(m)
                if m - LAG >= 0:
                    emit_y(m - LAG)
                if m < len(inj):
                    inj[m]()

        # ---- full schedule --------------------------------------------
        alloc_qk(0)
        for which in range(2):
            for half in range(2):
                emit_qk_group(0, which, half)

        for j in range(4):
            inject = []
            if j == 0:
                inject += [lambda tt=tt: emit_v_group(tt) for tt in range(8)]
            if j < 3:
                alloc_qk(j + 1)
                inject += [
                    lambda jj=j + 1, w=w, h=h: emit_qk_group(jj, w, h)
                    for w in range(2) for h in range(2)]
            if j == 2:
                dma(wp[:, :], wpt_d[:, :])
            emit_pair(j, inject)

        # ---- partial projection y_heads @ wproj_slice^T (bf16) --------
        for tt in range(8):
            ot = opool.tile([128, C], F32, tag="ot", name="ot_t")
            for nch in range(2):
                ps = pss.tile([128, 512], F32, tag="pss", name="ps_p")
                for jj in range(4):
                    nc.tensor.matmul(
                        ps[:, :],
                        yt_sb[jj][:, tt * 128:(tt + 1) * 128],
                        wps(jj)[:, nch * 512:(nch + 1) * 512],
                        start=(jj == 0), stop=(jj == 3),
                    )
                nc.scalar.copy(ot[:, nch * 512:(nch + 1) * 512], ps[:, :])
            dma(yp_d[tt * 128:(tt + 1) * 128, :], ot[:, :])

    nc.compile()   # bacc register allocation + finalize before serialization
    return nc


def _get_nc():
    if "nc" not in _CACHE:
        _CACHE["nc"] = build_nc()
    return _CACHE["nc"]


def _get_u():
    """Exactly reproduce the uniform field jax.random.bernoulli draws.

    The container pins jax_default_prng_impl='rbg', whose bitstream is
    backend-dependent — so this must run on the same default device the
    reference uses (do NOT pin to CPU here).
    """
    import jax
    import jax.numpy as jnp
    samp_key = jax.random.fold_in(jax.random.key(0), 42)
    u = jax.random.uniform(samp_key, (B, H, T, T), dtype=jnp.float32)
    return np.asarray(u)


def _prep_ltp():
    """Per-core packed causal logit thresholds: [NCORES, HPC, 128, 4608]."""
    if "ltp" in _CACHE:
        return _CACHE["ltp"]
    u = _get_u()
    ltp = np.empty((NCORES, HPC, 128, LT_COLS), np.float32)
    kk = np.arange(128)[:, None]
    for c in range(NCORES):
        b, hg = c // 2, c % 2
        for i in range(HPC):
            h = hg * HPC + i
            for (qc, kt, qs, n, boff) in LT_BLOCKS:
                ub = u[b, h, qs:qs + n, kt * 128:(kt + 1) * 128].astype(np.float64)
                with np.errstate(divide="ignore"):
                    lt = np.log(ub) - np.log1p(-ub)
                ltb = np.ascontiguousarray(lt.T).astype(np.float32)  # [128, n]
                if qs == kt * 128:
                    ltb[kk > np.arange(n)[None, :]] = np.inf
                ltp[c, i, :, boff:boff + n] = ltb
    _CACHE["ltp"] = ltp
    return ltp


def _pack_rows(a, k):
    """[k*128, W] -> [128, k*W]: partition p holds rows p, 128+p, ..."""
    kk, w = a.shape[0] // 128, a.shape[1]
    assert kk == k
    return np.ascontiguousarray(
        a.reshape(k, 128, w).transpose(1, 0, 2).reshape(128, k * w))


def make_in_maps(x, w_attn, w_proj):
    scale = np.float32(1.0 / np.sqrt(np.float32(HS)))   # 0.125, exact pow2
    ltp = _prep_ltp()
    in_maps = []
    for c in range(NCORES):
        b, hg = c // 2, c % 2
        r0 = hg * HPC * HS
        xt = np.ascontiguousarray(x[b].T)                          # [C, T]
        wqt = np.ascontiguousarray((w_attn[r0:r0 + 512, :] * scale).T)
        wkt = np.ascontiguousarray(w_attn[C + r0:C + r0 + 512, :].T)
        wvt = np.ascontiguousarray(w_attn[2 * C + r0:2 * C + r0 + 512, :].T)
        wpt = np.ascontiguousarray(w_proj[:, r0:r0 + 512].T)       # [512, C]
        in_maps.append({
            "xt": _pack_rows(xt, 8),
            "wqt": _pack_rows(wqt, 8),
            "wkt": _pack_rows(wkt, 8),
            "wvt": _pack_rows(wvt, 8).astype(ml_dtypes.bfloat16),
            "wpt": _pack_rows(wpt, 4).astype(ml_dtypes.bfloat16),
            "ltp": ltp[c],
        })
    return in_maps


def assemble(results):
    """Combine per-core outputs into full outputs."""
    y = np.zeros((B, T, C), np.float32)
    att_sum = np.zeros((B, H, T), np.float32)
    for c in range(NCORES):
        b, hg = c // 2, c % 2
        y[b] += results[c]["yp"]
        att_sum[b, hg * HPC:(hg + 1) * HPC, :] = results[c]["asum"]
    att_var = np.zeros((B, H, T), np.float32)
    return y, att_sum, att_var


def kernel(x, w_attn, w_proj, **run_kwargs):
    x = np.asarray(x, dtype=np.float32)
    w_attn = np.asarray(w_attn, dtype=np.float32)
    w_proj = np.asarray(w_proj, dtype=np.float32)
    nc = _get_nc()
    in_maps = make_in_maps(x, w_attn, w_proj)
    res = run_bass_kernel_spmd(nc, in_maps, core_ids=list(range(NCORES)),
                               **run_kwargs)
    out = assemble(res.results)
    _CACHE["last_result"] = res
    return out


# revision 14
# speedup vs baseline: 1.1581x; 1.0463x over previous
"""Trainium2 Bass kernel: causal self-attention with sigmoid attention and
Bernoulli straight-through sampling (nn_CausalSelfAttention_57002805953253).

Key facts this implementation exploits:

* In the forward pass, the straight-through estimator makes the attention
  matrix numerically EQUAL to the Bernoulli samples (att + (samp - att) ==
  samp exactly in fp32).  Therefore att_var == 0 exactly, att_sum is the
  per-row count of successes, and y is computed from the 0/1 samples.
* The Bernoulli draw is `u < sigmoid(scores)` with u = uniform(key, shape)
  for a FIXED key (input independent).  Since sigmoid is monotonic,
  `u < sigmoid(s)  <=>  s > logit(u)`, so the device kernel only needs the
  raw scores and a precomputed threshold tensor L = logit(u).  The causal
  mask is baked into L as +inf (s > +inf is always false).
* Sharding: core c handles batch b = c//2 and heads hg*8..hg*8+8 (hg=c%2).
  The final projection is computed per-core against the head-slice of
  w_proj; the host adds the two partial results per batch.

Precision strategy: scores (qkv q/k + q@k^T) run as float32r (fp32 data,
FP22 multiply, 2 cycles/row on this silicon) because sample flips scale
with score error.  The value path (v, samples @ v, projection) runs bf16
(1 cycle/row): samples are exactly representable there and the value-path
error is dominated by the sample flips anyway.

Performance notes (from NTFF traces):
* The PE executes its stream in order, so the kernel is emitted as one
  software-pipelined schedule: per causal block m, score-pair(m) then the
  DVE compare(m) then y-pair(m-2), with one qkv/v accumulation group
  injected per slot so the PE never idles on the compare/DMA latency and
  HAM stays un-throttled.
* DMA descriptors are one partition-row each and the DGE processes them
  serially (~130 GB/s/queue at small rows), so all large streams are
  host-packed partition-contiguous: x/w as [128, k*width] and the logit
  thresholds as one [128, 4608] tile per head (18 KB rows).  Issues are
  round-robined over the sync/scalar/gpsimd queues, per k-tile so the
  first qkv accumulations start as soon as the first tiles land.
* Even/odd heads' K=64 score matmuls use PE row-groups 0-63/64-127 and
  are emitted adjacently so they execute concurrently.
"""

import os
import sys
from contextlib import ExitStack

import numpy as np

for _p in ("/opt/trn_rl_repo", "/root/.axon_site/_ro/trn_rl_repo"):
    if os.path.isdir(_p) and _p not in sys.path:
        sys.path.append(_p)

import ml_dtypes  # noqa: E402
import concourse.bass as bass  # noqa: E402,F401
import concourse.tile as tile  # noqa: E402
from concourse import bacc, mybir  # noqa: E402
from concourse.bass_utils import run_bass_kernel_spmd  # noqa: E402

B, T, C, H = 4, 1024, 1024, 16
HS = C // H            # 64 head size
NCORES = 8
HPC = H // 2           # 8 local heads per core
F32 = mybir.dt.float32
F32R = mybir.dt.float32r
BF16 = mybir.dt.bfloat16


def _lt_blocks():
    """Flat causal block schedule: [(qc, kt, qs, n, boff)] + total columns."""
    blocks = []
    boff = 0
    for qc in range(2):
        for kt in range(4 if qc == 0 else 8):
            qs = max(kt * 128, qc * 512)
            n = qc * 512 + 512 - qs
            blocks.append((qc, kt, qs, n, boff))
            boff += n
    return blocks, boff


LT_BLOCKS, LT_COLS = _lt_blocks()   # 12 blocks, LT_COLS = 4608 columns/head

_CACHE = {}


def build_nc():
    """Build the SPMD Bass program (identical on all 8 cores)."""
    nc = bacc.Bacc("TRN2", target_bir_lowering=False, debug=False)

    # all big inputs host-packed partition-contiguous as [128, k*width]
    xt_d = nc.dram_tensor("xt", [128, 8 * T], F32R, kind="ExternalInput")
    wqt_d = nc.dram_tensor("wqt", [128, 8 * 512], F32R, kind="ExternalInput")
    wkt_d = nc.dram_tensor("wkt", [128, 8 * 512], F32R, kind="ExternalInput")
    wvt_d = nc.dram_tensor("wvt", [128, 8 * 512], BF16, kind="ExternalInput")
    wpt_d = nc.dram_tensor("wpt", [128, 4 * C], BF16, kind="ExternalInput")
    ltp_d = nc.dram_tensor("ltp", [HPC, 128, LT_COLS], F32, kind="ExternalInput")
    yp_d = nc.dram_tensor("yp", [T, C], F32, kind="ExternalOutput")
    asum_d = nc.dram_tensor("asum", [HPC, T], F32, kind="ExternalOutput")

    with tile.TileContext(nc) as tc, ExitStack() as ctx:
        persist = ctx.enter_context(tc.tile_pool(name="persist", bufs=1))
        qkpool = ctx.enter_context(tc.tile_pool(name="qkpool", bufs=1))
        ltp0 = ctx.enter_context(tc.tile_pool(name="ltp0", bufs=3))
        ltp1 = ctx.enter_context(tc.tile_pool(name="ltp1", bufs=3))
        smpool = ctx.enter_context(tc.tile_pool(name="smpool", bufs=8))
        opool = ctx.enter_context(tc.tile_pool(name="opool", bufs=2))
        astp = ctx.enter_context(tc.tile_pool(name="astp", bufs=2))
        psa = ctx.enter_context(tc.tile_pool(name="psa", bufs=2, space="PSUM"))
        pss = ctx.enter_context(tc.tile_pool(name="pss", bufs=4, space="PSUM"))
        psy = ctx.enter_context(tc.tile_pool(name="psy", bufs=2, space="PSUM"))

        engs = [nc.sync, nc.scalar, nc.gpsimd]
        rr = [0]

        def dma(dst, src):
            e = engs[rr[0] % 3]
            rr[0] += 1
            e.dma_start(dst, src)

        # ---- load packed x^T and weights, per k-tile ------------------
        xt = persist.tile([128, 8 * T], F32R, tag="xt", name="xt")
        xtb = persist.tile([128, 8 * T], BF16, tag="xtb", name="xtb")
        wq = persist.tile([128, 8 * 512], F32R, tag="wq", name="wq")
        wk = persist.tile([128, 8 * 512], F32R, tag="wk", name="wk")
        wv = persist.tile([128, 8 * 512], BF16, tag="wv", name="wv")
        wp = persist.tile([128, 4 * C], BF16, tag="wp", name="wp")

        for kc in range(8):
            dma(wv[:, kc * 512:(kc + 1) * 512],
                wvt_d[:, kc * 512:(kc + 1) * 512])
            dma(xt[:, kc * T:(kc + 1) * T], xt_d[:, kc * T:(kc + 1) * T])
            nc.scalar.copy(xtb[:, kc * T:(kc + 1) * T],
                           xt[:, kc * T:(kc + 1) * T])
            dma(wq[:, kc * 512:(kc + 1) * 512],
                wqt_d[:, kc * 512:(kc + 1) * 512])
            dma(wk[:, kc * 512:(kc + 1) * 512],
                wkt_d[:, kc * 512:(kc + 1) * 512])

        def xts(kc):
            return xt[:, kc * T:(kc + 1) * T]

        def xtbs(kc):
            return xtb[:, kc * T:(kc + 1) * T]

        def wqs(kc):
            return wq[:, kc * 512:(kc + 1) * 512]

        def wks(kc):
            return wk[:, kc * 512:(kc + 1) * 512]

        def wvs(kc):
            return wv[:, kc * 512:(kc + 1) * 512]

        def wps(jj):
            return wp[:, jj * C:(jj + 1) * C]

        # ---- PE warmup: dummy matmuls with no DMA deps so the HAM
        #      un-throttles while the initial loads stream in ------------
        wrm = persist.tile([128, 512], BF16, tag="wrm", name="wrm")
        nc.vector.memset(wrm[:, :], 0.0)
        for w_ in range(64):
            wps_ = psa.tile([128, 512], F32, tag="psa", name="ps_wrm")
            nc.tensor.matmul(wps_[:, :], wrm[:, 0:128], wrm[:, :],
                             start=True, stop=True)

        # ---- persistent result tiles ----------------------------------
        vaug = [persist.tile([128, HPC * (HS + 1)], BF16,
                             tag=f"va{t}", name=f"va{t}")
                for t in range(8)]
        yt_sb = [persist.tile([128, T], BF16, tag=f"yt{j}", name=f"yt{j}")
                 for j in range(4)]
        qk_tiles = {}

        # ---- PE work-group emitters (each = one accumulation group) ---
        def emit_v_group(tt):
            ps = psa.tile([128, 512], F32, tag="psa", name="ps_v")
            for kc in range(8):
                nc.tensor.matmul(
                    ps[:, :],
                    xtbs(kc)[:, tt * 128:(tt + 1) * 128],
                    wvs(kc),
                    start=(kc == 0), stop=(kc == 7),
                )
            nc.scalar.copy(
                vaug[tt][:, :].rearrange("p (h e) -> p h e", e=HS + 1)[:, :, 0:HS],
                ps[:, :].rearrange("p (h e) -> p h e", e=HS),
            )
            nc.vector.memset(
                vaug[tt][:, :].rearrange("p (h e) -> p h e", e=HS + 1)[:, :, HS:HS + 1],
                1.0,
            )

        def alloc_qk(j):
            qt = qkpool.tile([128, T], F32R, tag=f"qt{j % 2}", name=f"qt{j}")
            kt = qkpool.tile([128, T], F32R, tag=f"kt{j % 2}", name=f"kt{j}")
            qk_tiles[j] = (qt, kt)

        def emit_qk_group(j, which, half):
            qt, kt = qk_tiles[j]
            dst, wview = (qt, wqs) if which == 0 else (kt, wks)
            ps = psa.tile([128, 512], F32, tag="psa", name="ps_qk")
            for kc in range(8):
                nc.tensor.matmul(
                    ps[:, :],
                    wview(kc)[:, j * 128:(j + 1) * 128],
                    xts(kc)[:, half * 512:(half + 1) * 512],
                    start=(kc == 0), stop=(kc == 7),
                )
            nc.scalar.copy(dst[:, half * 512:(half + 1) * 512], ps[:, :])

        # ---- software-pipelined attention for head-pair j -------------
        LAG = 2

        def emit_pair(j, inject):
            """inject: list of callables; at most one is emitted per slot."""
            qt, kt_t = qk_tiles[j]
            lth = {}            # (par, qc) -> (tile, col_base)
            for par in range(2):
                i = 2 * j + par
                a = ltp0.tile([128, 1280], F32, tag="lt0", name=f"lt0_{i}")
                dma(a[:, :], ltp_d[i, :, 0:1280])
                lth[(par, 0)] = (a, 0)
            for par in range(2):
                i = 2 * j + par
                b = ltp1.tile([128, LT_COLS - 1280], F32, tag="lt1",
                              name=f"lt1_{i}")
                dma(b[:, :], ltp_d[i, :, 1280:LT_COLS])
                lth[(par, 1)] = (b, 1280)
            nblocks = len(LT_BLOCKS)          # 12
            yps_by_qc = {}
            smp_tiles = [None] * nblocks      # per block: (smp_e, smp_o)
            inj = list(inject)

            def qc_of(m):
                return LT_BLOCKS[m][0]

            def emit_score_and_compare(m):
                (qc, kt, qs, n, boff) = LT_BLOCKS[m]
                if qc not in yps_by_qc:
                    yps_by_qc[qc] = [
                        psy.tile([HS + 1, 512], F32, tag="psy", name="psy_e"),
                        psy.tile([HS + 1, 512], F32, tag="psy", name="psy_o")]
                sps = []
                for par in range(2):
                    po = 64 * par
                    sp = pss.tile([128, n], F32, tag="pss", name="ps_s")
                    nc.tensor.matmul(
                        sp[:, :],
                        kt_t[po:po + 64, kt * 128:(kt + 1) * 128],
                        qt[po:po + 64, qs:qs + n],
                        start=True, stop=True,
                    )
                    sps.append(sp)
                pair_smp = []
                for par in range(2):
                    lt_t, cbase = lth[(par, qc)]
                    smp = smpool.tile([128, n], BF16, tag="smp", name="smp_t")
                    nc.vector.tensor_tensor(
                        smp[:, :], sps[par][:, :],
                        lt_t[:, boff - cbase:boff - cbase + n],
                        op=mybir.AluOpType.is_gt,
                    )
                    pair_smp.append(smp)
                smp_tiles[m] = pair_smp

            def emit_y(m):
                (qc, kt, qs, n, boff) = LT_BLOCKS[m]
                first_kt = 0
                last_kt = 3 if qc == 0 else 7
                o = qs - qc * 512
                for par in range(2):
                    i = 2 * j + par
                    nc.tensor.matmul(
                        yps_by_qc[qc][par][:, o:o + n],
                        vaug[kt][:, i * (HS + 1):(i + 1) * (HS + 1)],
                        smp_tiles[m][par][:, :],
                        start=(kt == first_kt), stop=(kt == last_kt),
                    )
                if kt == last_kt:             # qc complete -> drain psums
                    for par in range(2):
                        i = 2 * j + par
                        po = 64 * par
                        nc.scalar.copy(
                            yt_sb[j][po:po + 64, qc * 512:(qc + 1) * 512],
                            yps_by_qc[qc][par][0:64, :])
                        ast = astp.tile([65, 512], F32, tag="ast",
                                        name="ast_t")
                        nc.scalar.copy(ast[64:65, :],
                                       yps_by_qc[qc][par][64:65, :])
                        dma(asum_d[i, qc * 512:(qc + 1) * 512], ast[64:65, :])

            for m in range(nblocks + LAG):
                if m < nblocks:
                    emit_score_and_compare(m)
                if m - LAG >= 0:
                    emit_y(m - LAG)
                if m < len(inj):
                    inj[m]()

        # ---- full schedule --------------------------------------------
        alloc_qk(0)
        for half in range(2):
            for which in range(2):
                emit_qk_group(0, which, half)

        for j in range(4):
            inject = []
            if j == 0:
                inject += [lambda tt=tt: emit_v_group(tt) for tt in range(8)]
            if j < 3:
                alloc_qk(j + 1)
                inject += [
                    lambda jj=j + 1, w=w, h=h: emit_qk_group(jj, w, h)
                    for h in range(2) for w in range(2)]
            if j == 2:
                dma(wp[:, :], wpt_d[:, :])
            emit_pair(j, inject)

        # ---- partial projection y_heads @ wproj_slice^T (bf16) --------
        for tt in range(8):
            ot = opool.tile([128, C], F32, tag="ot", name="ot_t")
            for nch in range(2):
                ps = pss.tile([128, 512], F32, tag="pss", name="ps_p")
                for jj in range(4):
                    nc.tensor.matmul(
                        ps[:, :],
                        yt_sb[jj][:, tt * 128:(tt + 1) * 128],
                        wps(jj)[:, nch * 512:(nch + 1) * 512],
                        start=(jj == 0), stop=(jj == 3),
                    )
                nc.scalar.copy(ot[:, nch * 512:(nch + 1) * 512], ps[:, :])
            dma(yp_d[tt * 128:(tt + 1) * 128, :], ot[:, :])

    nc.compile()   # bacc register allocation + finalize before serialization
    return nc


def _get_nc():
    if "nc" not in _CACHE:
        _CACHE["nc"] = build_nc()
    return _CACHE["nc"]


def _get_u():
    """Exactly reproduce the uniform field jax.random.bernoulli draws.

    The container pins jax_default_prng_impl='rbg', whose bitstream is
    backend-dependent — so this must run on the same default device the
    reference uses (do NOT pin to CPU here).
    """
    import jax
    import jax.numpy as jnp
    samp_key = jax.random.fold_in(jax.random.key(0), 42)
    u = jax.random.uniform(samp_key, (B, H, T, T), dtype=jnp.float32)
    return np.asarray(u)


def _prep_ltp():
    """Per-core packed causal logit thresholds: [NCORES, HPC, 128, 4608]."""
    if "ltp" in _CACHE:
        return _CACHE["ltp"]
    u = _get_u()
    ltp = np.empty((NCORES, HPC, 128, LT_COLS), np.float32)
    kk = np.arange(128)[:, None]
    for c in range(NCORES):
        b, hg = c // 2, c % 2
        for i in range(HPC):
            h = hg * HPC + i
            for (qc, kt, qs, n, boff) in LT_BLOCKS:
                ub = u[b, h, qs:qs + n, kt * 128:(kt + 1) * 128].astype(np.float64)
                with np.errstate(divide="ignore"):
                    lt = np.log(ub) - np.log1p(-ub)
                ltb = np.ascontiguousarray(lt.T).astype(np.float32)  # [128, n]
                if qs == kt * 128:
                    ltb[kk > np.arange(n)[None, :]] = np.inf
                ltp[c, i, :, boff:boff + n] = ltb
    _CACHE["ltp"] = ltp
    return ltp


def _pack_rows(a, k):
    """[k*128, W] -> [128, k*W]: partition p holds rows p, 128+p, ..."""
    kk, w = a.shape[0] // 128, a.shape[1]
    assert kk == k
    return np.ascontiguousarray(
        a.reshape(k, 128, w).transpose(1, 0, 2).reshape(128, k * w))


def make_in_maps(x, w_attn, w_proj):
    scale = np.float32(1.0 / np.sqrt(np.float32(HS)))   # 0.125, exact pow2
    ltp = _prep_ltp()
    in_maps = []
    for c in range(NCORES):
        b, hg = c // 2, c % 2
        r0 = hg * HPC * HS
        xt = np.ascontiguousarray(x[b].T)                          # [C, T]
        wqt = np.ascontiguousarray((w_attn[r0:r0 + 512, :] * scale).T)
        wkt = np.ascontiguousarray(w_attn[C + r0:C + r0 + 512, :].T)
        wvt = np.ascontiguousarray(w_attn[2 * C + r0:2 * C + r0 + 512, :].T)
        wpt = np.ascontiguousarray(w_proj[:, r0:r0 + 512].T)       # [512, C]
        in_maps.append({
            "xt": _pack_rows(xt, 8),
            "wqt": _pack_rows(wqt, 8),
            "wkt": _pack_rows(wkt, 8),
            "wvt": _pack_rows(wvt, 8).astype(ml_dtypes.bfloat16),
            "wpt": _pack_rows(wpt, 4).astype(ml_dtypes.bfloat16),
            "ltp": ltp[c],
        })
    return in_maps


def assemble(results):
    """Combine per-core outputs into full outputs."""
    y = np.zeros((B, T, C), np.float32)
    att_sum = np.zeros((B, H, T), np.float32)
    for c in range(NCORES):
        b, hg = c // 2, c % 2
        y[b] += results[c]["yp"]
        att_sum[b, hg * HPC:(hg + 1) * HPC, :] = results[c]["asum"]
    att_var = np.zeros((B, H, T), np.float32)
    return y, att_sum, att_var


def kernel(x, w_attn, w_proj, **run_kwargs):
    x = np.asarray(x, dtype=np.float32)
    w_attn = np.asarray(w_attn, dtype=np.float32)
    w_proj = np.asarray(w_proj, dtype=np.float32)
    nc = _get_nc()
    in_maps = make_in_maps(x, w_attn, w_proj)
    res = run_bass_kernel_spmd(nc, in_maps, core_ids=list(range(NCORES)),
                               **run_kwargs)
    out = assemble(res.results)
    _CACHE["last_result"] = res
    return out
